# revision 20
# baseline (speedup 1.0000x reference)
"""Trainium2 Bass kernel for column-softmax attention.

reference semantics:
    scores = einsum('bqd,bkd->bqk', q, k) / sqrt(128)   # [B, Nq, Nk]
    attn   = softmax(scores, axis=1)                     # over the QUERY axis
    out    = einsum('bqk,bkd->bqd', attn, v)             # [B, Nq, D]

Because the softmax is over q, each key column k normalizes independently:
    out[q, d] = sum_k E[k, q] * r[k] * v[k, d],  E = exp(scores.T), r = 1/sum_q E[k, q]

Sharding: 8 cores = 4 batches x 2 key-halves.  Each core computes the partial
sum over its 2048 keys; the host adds the two partials per batch.

On-chip layout: the host pre-transposes Q and K to [D, N] (contraction dim on
partitions) and the kernel emits out.T [D, Nq]; the host transposes back.  The
softmax denominator is folded into V row-scaling so the normalize step touches
only 128x128 tiles per key tile.

The ScalarE exp pass (8.4M elements/core) is the roofline: 64 ACTIVATE
instructions of N=1024 from PSUM = ~66us engine-busy, the largest chunk the
8-bank PSUM allows while keeping a double-buffered score tile (4 banks) AND a
query-half output accumulator (4 banks) resident.  Structure:
  - row sums moved OFF ScalarE (accum_out costs ~190-280ns per ACTIVATE):
    per key tile a 3-stage fp16 pairwise-add chain + short reduce on DVE
    (~2.9us, fits under the 4.15us ACT tile period).  Last tile keeps
    accum_out so vsc15 is ready immediately for the tail.
  - startup: first q chunk + first key tile DMA'd first; a dummy exp preloads
    the ACT table and a 5-matmul memset warm-up stream releases the PE HAM
    clock-gate, all under the initial DMA window.
  - phase A (per key tile): scores matmul -> exp -> E resident fp16; previous
    tile's out.T contribution for query half A accumulated in PSUM (hides the
    second GEMM under the ScalarE exp span).
  - tail: query half B accumulated in the freed S-pool PSUM banks (so it does
    not wait on the half-A flush); half-A close ordered before the half-B
    stream; flush copies in 512-col quarters split ACT/DVE, fp16 staging
    halves the out DMA, each quarter DMAs as soon as staged.
PE weight-load runs are kept contiguous per stationary via explicit
ordering-only deps (the scheduler would otherwise split runs).
Measured (TimelineSim / axon loop-difference): 110.8us/95.3us baseline ->
92.9us model, ~79-81us hardware.
"""

import numpy as np

import concourse.bass as bass
import concourse.mybir as mybir
import concourse.tile as tile
from concourse.bass_utils import run_bass_kernel_spmd
from concourse.tile_rust import add_dep_helper

B, N, D = 4, 4096, 128
P = 128
NK = 2048                 # keys per core (half of 4096)
KT_TILES = NK // P        # 16 key tiles of 128
SCALE = 1.0 / np.sqrt(128.0)

F32 = mybir.dt.float32
F16 = mybir.dt.float16


def emit_body(nc, tc, pools, aps):
    big, epool, small, spsum, opsum = pools
    qt_d, kt_d, v_d, out_d = aps

    qT = big.tile([P, N], F16, tag="qT")            # [d, q]
    kT = big.tile([P, NK], F16, tag="kT")           # [d, k]
    vsb = big.tile([P, KT_TILES, D], F16, tag="v")  # [k_in_tile, k_tile, d]
    oacc = big.tile([P, N], F16, tag="oacc")        # [d, q] fp16 staging

    # DMA order: the first scores matmul needs kT tile 0 + qT[0:512] only.
    nc.sync.dma_start(qT[:, 0:512], qt_d[:, 0:512])
    nc.sync.dma_start(kT[:, 0:P], kt_d[:, 0:P])
    nc.sync.dma_start(qT[:, 512:1024], qt_d[:, 512:1024])
    for _qc in range(1, 4):
        nc.sync.dma_start(
            qT[:, _qc * 1024 : (_qc + 1) * 1024], qt_d[:, _qc * 1024 : (_qc + 1) * 1024]
        )
    # kT tiles 1.. are first needed at tile-1 scores (~7us in), q chunks first.
    nc.sync.dma_start(kT[:, P:NK], kt_d[:, P:NK])
    nc.sync.dma_start(vsb[:], v_d.rearrange("(t p) d -> p t d", p=P))

    # ACT table preload: dummy exp on a memset tile runs under the DMA window,
    # so the ~1.3us exp_and_others table load is off the critical path.
    warm_in = small.tile([P, 1], F32, tag="warm_in")
    warm_out = small.tile([P, 1], F32, tag="warm_out")
    nc.vector.memset(warm_in[:], 0.0)
    nc.scalar.activation(
        out=warm_out[:], in_=warm_in[:], func=mybir.ActivationFunctionType.Exp
    )

    # PE warm-up: a stream of matmuls on memset data runs during the
    # otherwise-idle DMA window, so the HAM clock-gate (cold 1.2GHz ->
    # warm 2.4GHz after ~3.4us of sustained PE activity) is released
    # before the first real scores matmul arrives.
    wsrc = big.tile([P, 512], F16, tag="wsrc")
    nc.vector.memset(wsrc[:], 0.0)
    Swarm = spsum.tile([P, 1024], F32, tag="S")
    for _w in range(5):
        nc.tensor.matmul(
            Swarm[:, 0:512], lhsT=wsrc[:, 0:P], rhs=wsrc[:], start=True, stop=True
        )

    e_tiles = []
    v_tiles = []
    # Output accumulators for query half A (cols 0..2047) are built up during
    # phase A so the second GEMM's first half hides under the exp span.
    oa_tiles = []
    for _oc in range(2):
        O_a = opsum.tile([P, 1024], F32, tag="O")
        oa_tiles.append(O_a)

    pending_g2a = None
    for kt in range(KT_TILES):
        last_tile = kt == KT_TILES - 1
        E = epool.tile([P, N], F16, tag=f"E{kt}")   # [k, q] = exp(scores.T)
        rs = small.tile([P, 4], F32, tag="rs")
        last_g1 = None
        for h in range(4):
            S = spsum.tile([P, 1024], F32, tag="S")
            for u in range(2):
                last_g1 = nc.tensor.matmul(
                    S[:, u * 512 : (u + 1) * 512],
                    lhsT=kT[:, kt * P : (kt + 1) * P],
                    rhs=qT[:, h * 1024 + u * 512 : h * 1024 + u * 512 + 512],
                    start=True,
                    stop=True,
                )
            # For the LAST tile only, fuse the row sums into the activation
            # (accum_out): the DVE reduce chain would otherwise put ~3us of
            # vsc15 latency on the critical path into the tail.  For all
            # other tiles the fused accumulator read costs ~190-280ns of
            # ScalarE per chunk, so the row sums go to DVE instead.
            nc.scalar.activation(
                out=E[:, h * 1024 : (h + 1) * 1024],
                in_=S[:],
                func=mybir.ActivationFunctionType.Exp,
                scale=float(SCALE),
                accum_out=rs[:, h : h + 1] if last_tile else None,
            )
        rsum = small.tile([P, 1], F32, tag="rsum")
        recip = small.tile([P, 1], F32, tag="recip")
        vsc = small.tile([P, D], F16, tag=f"vsc{kt}")  # [k, d] * r[k]
        if last_tile:
            nc.vector.reduce_sum(out=rsum[:], in_=rs[:], axis=mybir.AxisListType.X)
        else:
            # Row sums on DVE, off the ScalarE path.  A straight TensorReduce
            # over [128, 4096] runs at 1 elem/cycle = ~4.3us/tile and would
            # make DVE the per-tile bottleneck; fp16 pairwise adds get the
            # 2x DVE rate, so fold 4096 -> 512 in three adds + short reduce
            # (~2.9us).  (tensor_tensor_reduce would be one instruction but
            # does not compile on this toolchain: "ISA wrong length".)
            t1 = small.tile([P, 2048], F16, tag="rt1")
            t2 = small.tile([P, 1024], F16, tag="rt2")
            t3 = small.tile([P, 512], F16, tag="rt3")
            nc.vector.tensor_add(t1[:], E[:, 0:2048], E[:, 2048:4096])
            nc.vector.tensor_add(t2[:], t1[:, 0:1024], t1[:, 1024:2048])
            nc.vector.tensor_add(t3[:], t2[:, 0:512], t2[:, 512:1024])
            nc.vector.reduce_sum(out=rsum[:], in_=t3[:], axis=mybir.AxisListType.X)
        nc.vector.reciprocal(recip[:], rsum[:])
        nc.vector.tensor_scalar_mul(vsc[:], vsb[:, kt, :], recip[:])
        e_tiles.append(E)
        v_tiles.append(vsc)

        # Emit the PREVIOUS key tile's half-A output matmuls here, ordered
        # after this tile's scores matmuls (ordering-only deps).  This keeps
        # each kT weight-load run contiguous.
        if kt > 0:
            pv, pe_t, pkt = pending_g2a
            for oc in range(2):
                for u in range(2):
                    mm = nc.tensor.matmul(
                        oa_tiles[oc][:, u * 512 : (u + 1) * 512],
                        lhsT=pv[:],
                        rhs=pe_t[:, oc * 1024 + u * 512 : oc * 1024 + (u + 1) * 512],
                        start=(pkt == 0),
                        stop=False,
                    )
                    if last_g1 is not None:
                        add_dep_helper(
                            mm.ins,
                            last_g1.ins,
                            sync=False,
                            reason="keep kT weight-load run contiguous",
                        )
        pending_g2a = (vsc, E, kt)

    # Close the half-A accumulation with the last key tile's contribution.
    pv, pe_t, pkt = pending_g2a
    last_close = None
    for oc in range(2):
        for u in range(2):
            last_close = nc.tensor.matmul(
                oa_tiles[oc][:, u * 512 : (u + 1) * 512],
                lhsT=pv[:],
                rhs=pe_t[:, oc * 1024 + u * 512 : oc * 1024 + (u + 1) * 512],
                start=False,
                stop=True,
            )

    # Tail: query half B accumulates in the S-pool banks (free once the last
    # exp has read them) so it does NOT wait on the half-A flush below.
    ob_tiles = []
    for _oc in range(2):
        O_b = spsum.tile([P, 1024], F32, tag="S")
        ob_tiles.append(O_b)
    prev_mm = None
    for kt in range(KT_TILES):
        for oc in range(2):
            for u in range(2):
                mm = nc.tensor.matmul(
                    ob_tiles[oc][:, u * 512 : (u + 1) * 512],
                    lhsT=v_tiles[kt][:],
                    rhs=e_tiles[kt][:, 2048 + oc * 1024 + u * 512 : 2048 + oc * 1024 + (u + 1) * 512],
                    start=(kt == 0),
                    stop=(kt == KT_TILES - 1),
                )
                # chain ordering so each vsc weight-load run stays a
                # contiguous block of 4, and so the half-A close runs FIRST
                # (otherwise the scheduler defers it to the very end and the
                # half-A flush + DMAs pile up behind the half-B tail)
                add_dep_helper(
                    mm.ins,
                    (prev_mm or last_close).ins,
                    sync=False,
                    reason="contiguous vsc weight runs in tail",
                )
                prev_mm = mm

    # Flushes: fp16 staging (halves the out DMA bytes; the host adds the two
    # per-batch partials in fp32).  Copies split across ACT (idle in the
    # tail) and DVE in 512-col quarters so the last copy->DMA chain is short;
    # each quarter's DMA issues as soon as it is staged.
    def flush(o_pair, lo):
        for oc in range(2):
            for qr in range(2):
                src = o_pair[oc][:, qr * 512 : (qr + 1) * 512]
                dst = oacc[:, lo + oc * 1024 + qr * 512 : lo + oc * 1024 + (qr + 1) * 512]
                if oc == 0:
                    nc.scalar.copy(out=dst, in_=src)
                else:
                    nc.vector.tensor_copy(out=dst, in_=src)
                nc.sync.dma_start(
                    out_d[:, lo + oc * 1024 + qr * 512 : lo + oc * 1024 + (qr + 1) * 512],
                    dst,
                )

    flush(oa_tiles, 0)
    flush(ob_tiles, 2048)


def build_bass(repeat=1, loop=False):
    nc = bass.Bass("TRN2", target_bir_lowering=False, debug=False)
    qt_d = nc.dram_tensor("qt", [P, N], F16, kind="ExternalInput").ap()
    kt_d = nc.dram_tensor("kt", [P, NK], F16, kind="ExternalInput").ap()
    v_d = nc.dram_tensor("v", [NK, D], F16, kind="ExternalInput").ap()
    out_d = nc.dram_tensor("out_t", [P, N], F16, kind="ExternalOutput").ap()

    with tile.TileContext(nc) as tc:
        with (
            tc.tile_pool(name="big", bufs=1) as big,
            tc.tile_pool(name="epool", bufs=1) as epool,
            tc.tile_pool(name="small", bufs=2) as small,
            tc.tile_pool(name="spsum", bufs=2, space="PSUM") as spsum,
            tc.tile_pool(name="opsum", bufs=2, space="PSUM") as opsum,
        ):
            def body():
                emit_body(
                    nc,
                    tc,
                    (big, epool, small, spsum, opsum),
                    (qt_d, kt_d, v_d, out_d),
                )

            if loop and repeat > 1:
                with tc.For_i(
                    0, repeat, 1,
                    hint_engines=(mybir.EngineType.PE, mybir.EngineType.Activation),
                ):
                    body()
            else:
                for _ in range(repeat):
                    body()
    return nc


def legalize_waits(nc, max_waits=1):
    """Hoist excess semaphore waits into standalone EventSemaphore ops.

    The walrus codegen for several engine instruction structs accepts only a
    single sync-wait command; Tile sometimes emits more.  Executing the extra
    waits in a preceding same-engine EventSemaphore is semantically identical
    (the engine runs its stream in order).
    """
    for fn in nc.m.functions:
        for blk in fn.blocks:
            out = []
            for inst in blk.instructions:
                si = inst.sync_info
                if (
                    si is not None
                    and si.on_wait
                    and len(si.on_wait) > max_waits
                    and inst.opcode != "EventSemaphore"
                ):
                    waits = list(si.on_wait)
                    extra, keep = waits[:-max_waits], waits[-max_waits:]
                    for n, w in enumerate(extra):
                        out.append(
                            mybir.InstEventSemaphore(
                                name=f"{inst.name}_prewait{n}",
                                engine=inst.engine,
                                ins=[],
                                outs=[],
                                sync_info=mybir.SyncInfo(on_wait=[w], on_update=[]),
                            )
                        )
                    si.on_wait = keep
                out.append(inst)
            blk.instructions = out
    return nc


_NC_CACHE = {}


def _get_nc(repeat=1, **kw):
    key = ("nc", repeat, tuple(sorted(kw.items())))
    if key not in _NC_CACHE:
        _NC_CACHE[key] = legalize_waits(build_bass(repeat, **kw))
    return _NC_CACHE[key]


def kernel(q, k, v):
    q = np.asarray(q, dtype=np.float32)
    k = np.asarray(k, dtype=np.float32)
    v = np.asarray(v, dtype=np.float32)

    in_maps = []
    for c in range(8):
        b, h = c // 2, c % 2
        in_maps.append(
            {
                "qt": np.ascontiguousarray(q[b].T).astype(np.float16),
                "kt": np.ascontiguousarray(k[b, h * NK : (h + 1) * NK].T).astype(np.float16),
                "v": np.ascontiguousarray(v[b, h * NK : (h + 1) * NK]).astype(np.float16),
            }
        )

    nc = _get_nc()
    res = run_bass_kernel_spmd(nc, in_maps, list(range(8))).results

    out = np.empty((B, N, D), dtype=np.float32)
    for b in range(B):
        out[b] = (
            res[2 * b]["out_t"].astype(np.float32)
            + res[2 * b + 1]["out_t"].astype(np.float32)
        ).T
    return out


# revision 21
# speedup vs baseline: 1.0181x; 1.0181x over previous
"""Trainium2 Bass kernel for column-softmax attention.

reference semantics:
    scores = einsum('bqd,bkd->bqk', q, k) / sqrt(128)   # [B, Nq, Nk]
    attn   = softmax(scores, axis=1)                     # over the QUERY axis
    out    = einsum('bqk,bkd->bqd', attn, v)             # [B, Nq, D]

Because the softmax is over q, each key column k normalizes independently:
    out[q, d] = sum_k E[k, q] * r[k] * v[k, d],  E = exp(scores.T), r = 1/sum_q E[k, q]

Sharding: 8 cores = 4 batches x 2 key-halves.  Each core computes the partial
sum over its 2048 keys; the host adds the two partials per batch.

On-chip layout: the host pre-transposes Q and K to [D, N] (contraction dim on
partitions) and the kernel emits out.T [D, Nq]; the host transposes back.  The
softmax denominator is folded into V row-scaling so the normalize step touches
only 128x128 tiles per key tile.

The ScalarE exp pass (8.4M elements/core) is the roofline: 64 ACTIVATE
instructions of N=1024 from PSUM = ~66us engine-busy, the largest chunk the
8-bank PSUM allows while keeping a double-buffered score tile (4 banks) AND a
query-half output accumulator (4 banks) resident.  Structure:
  - row sums moved OFF ScalarE (accum_out costs ~190-280ns per ACTIVATE):
    per key tile a 3-stage fp16 pairwise-add chain + short reduce on DVE
    (~2.9us, fits under the 4.15us ACT tile period).  Last tile keeps
    accum_out so vsc15 is ready immediately for the tail.
  - startup: first q chunk + first key tile DMA'd first; a dummy exp preloads
    the ACT table and a 5-matmul memset warm-up stream releases the PE HAM
    clock-gate, all under the initial DMA window.
  - phase A (per key tile): scores matmul -> exp -> E resident fp16; previous
    tile's out.T contribution for query half A accumulated in PSUM (hides the
    second GEMM under the ScalarE exp span).
  - tail: query half B accumulated in the freed S-pool PSUM banks (so it does
    not wait on the half-A flush); half-A close ordered before the half-B
    stream; flush copies in 512-col quarters split ACT/DVE, fp16 staging
    halves the out DMA, each quarter DMAs as soon as staged.
PE weight-load runs are kept contiguous per stationary via explicit
ordering-only deps (the scheduler would otherwise split runs).
Measured: TimelineSim 110.8us baseline -> 92.9us.  Axon loop-difference
(back-to-back A/B, R=257 sustained): baseline 110.1us -> 94.6us; quiet-window
R=129 burst readings ~79-81us vs the 95.3us baseline quote (~14-16% either way).
"""

import numpy as np

import concourse.bass as bass
import concourse.mybir as mybir
import concourse.tile as tile
from concourse.bass_utils import run_bass_kernel_spmd
from concourse.tile_rust import add_dep_helper

B, N, D = 4, 4096, 128
P = 128
NK = 2048                 # keys per core (half of 4096)
KT_TILES = NK // P        # 16 key tiles of 128
SCALE = 1.0 / np.sqrt(128.0)

F32 = mybir.dt.float32
F16 = mybir.dt.float16


def emit_body(nc, tc, pools, aps):
    big, epool, small, spsum, opsum = pools
    qt_d, kt_d, v_d, out_d = aps

    qT = big.tile([P, N], F16, tag="qT")            # [d, q]
    kT = big.tile([P, NK], F16, tag="kT")           # [d, k]
    vsb = big.tile([P, KT_TILES, D], F16, tag="v")  # [k_in_tile, k_tile, d]
    oacc = big.tile([P, N], F16, tag="oacc")        # [d, q] fp16 staging

    # DMA order: the first scores matmul needs kT tile 0 + qT[0:512] only.
    nc.sync.dma_start(qT[:, 0:512], qt_d[:, 0:512])
    nc.sync.dma_start(kT[:, 0:P], kt_d[:, 0:P])
    nc.sync.dma_start(qT[:, 512:1024], qt_d[:, 512:1024])
    for _qc in range(1, 4):
        nc.sync.dma_start(
            qT[:, _qc * 1024 : (_qc + 1) * 1024], qt_d[:, _qc * 1024 : (_qc + 1) * 1024]
        )
    # kT tiles 1.. are first needed at tile-1 scores (~7us in), q chunks first.
    nc.sync.dma_start(kT[:, P:NK], kt_d[:, P:NK])
    nc.sync.dma_start(vsb[:], v_d.rearrange("(t p) d -> p t d", p=P))

    # ACT table preload: dummy exp on a memset tile runs under the DMA window,
    # so the ~1.3us exp_and_others table load is off the critical path.
    warm_in = small.tile([P, 1], F32, tag="warm_in")
    warm_out = small.tile([P, 1], F32, tag="warm_out")
    nc.vector.memset(warm_in[:], 0.0)
    nc.scalar.activation(
        out=warm_out[:], in_=warm_in[:], func=mybir.ActivationFunctionType.Exp
    )

    # PE warm-up: a stream of matmuls on memset data runs during the
    # otherwise-idle DMA window, so the HAM clock-gate (cold 1.2GHz ->
    # warm 2.4GHz after ~3.4us of sustained PE activity) is released
    # before the first real scores matmul arrives.
    wsrc = big.tile([P, 512], F16, tag="wsrc")
    nc.vector.memset(wsrc[:], 0.0)
    Swarm = spsum.tile([P, 1024], F32, tag="S")
    for _w in range(5):
        nc.tensor.matmul(
            Swarm[:, 0:512], lhsT=wsrc[:, 0:P], rhs=wsrc[:], start=True, stop=True
        )

    e_tiles = []
    v_tiles = []
    # Output accumulators for query half A (cols 0..2047) are built up during
    # phase A so the second GEMM's first half hides under the exp span.
    oa_tiles = []
    for _oc in range(2):
        O_a = opsum.tile([P, 1024], F32, tag="O")
        oa_tiles.append(O_a)

    pending_g2a = None
    for kt in range(KT_TILES):
        last_tile = kt == KT_TILES - 1
        E = epool.tile([P, N], F16, tag=f"E{kt}")   # [k, q] = exp(scores.T)
        rs = small.tile([P, 4], F32, tag="rs")
        last_g1 = None
        for h in range(4):
            S = spsum.tile([P, 1024], F32, tag="S")
            for u in range(2):
                last_g1 = nc.tensor.matmul(
                    S[:, u * 512 : (u + 1) * 512],
                    lhsT=kT[:, kt * P : (kt + 1) * P],
                    rhs=qT[:, h * 1024 + u * 512 : h * 1024 + u * 512 + 512],
                    start=True,
                    stop=True,
                )
            # For the LAST tile only, fuse the row sums into the activation
            # (accum_out): the DVE reduce chain would otherwise put ~3us of
            # vsc15 latency on the critical path into the tail.  For all
            # other tiles the fused accumulator read costs ~190-280ns of
            # ScalarE per chunk, so the row sums go to DVE instead.
            nc.scalar.activation(
                out=E[:, h * 1024 : (h + 1) * 1024],
                in_=S[:],
                func=mybir.ActivationFunctionType.Exp,
                scale=float(SCALE),
                accum_out=rs[:, h : h + 1] if last_tile else None,
            )
        rsum = small.tile([P, 1], F32, tag="rsum")
        recip = small.tile([P, 1], F32, tag="recip")
        vsc = small.tile([P, D], F16, tag=f"vsc{kt}")  # [k, d] * r[k]
        if last_tile:
            nc.vector.reduce_sum(out=rsum[:], in_=rs[:], axis=mybir.AxisListType.X)
        else:
            # Row sums on DVE, off the ScalarE path.  A straight TensorReduce
            # over [128, 4096] runs at 1 elem/cycle = ~4.3us/tile and would
            # make DVE the per-tile bottleneck; fp16 pairwise adds get the
            # 2x DVE rate, so fold 4096 -> 512 in three adds + short reduce
            # (~2.9us).  (tensor_tensor_reduce would be one instruction but
            # does not compile on this toolchain: "ISA wrong length".)
            t1 = small.tile([P, 2048], F16, tag="rt1")
            t2 = small.tile([P, 1024], F16, tag="rt2")
            t3 = small.tile([P, 512], F16, tag="rt3")
            nc.vector.tensor_add(t1[:], E[:, 0:2048], E[:, 2048:4096])
            nc.vector.tensor_add(t2[:], t1[:, 0:1024], t1[:, 1024:2048])
            nc.vector.tensor_add(t3[:], t2[:, 0:512], t2[:, 512:1024])
            nc.vector.reduce_sum(out=rsum[:], in_=t3[:], axis=mybir.AxisListType.X)
        nc.vector.reciprocal(recip[:], rsum[:])
        nc.vector.tensor_scalar_mul(vsc[:], vsb[:, kt, :], recip[:])
        e_tiles.append(E)
        v_tiles.append(vsc)

        # Emit the PREVIOUS key tile's half-A output matmuls here, ordered
        # after this tile's scores matmuls (ordering-only deps).  This keeps
        # each kT weight-load run contiguous.
        if kt > 0:
            pv, pe_t, pkt = pending_g2a
            for oc in range(2):
                for u in range(2):
                    mm = nc.tensor.matmul(
                        oa_tiles[oc][:, u * 512 : (u + 1) * 512],
                        lhsT=pv[:],
                        rhs=pe_t[:, oc * 1024 + u * 512 : oc * 1024 + (u + 1) * 512],
                        start=(pkt == 0),
                        stop=False,
                    )
                    if last_g1 is not None:
                        add_dep_helper(
                            mm.ins,
                            last_g1.ins,
                            sync=False,
                            reason="keep kT weight-load run contiguous",
                        )
        pending_g2a = (vsc, E, kt)

    # Close the half-A accumulation with the last key tile's contribution.
    pv, pe_t, pkt = pending_g2a
    last_close = None
    for oc in range(2):
        for u in range(2):
            last_close = nc.tensor.matmul(
                oa_tiles[oc][:, u * 512 : (u + 1) * 512],
                lhsT=pv[:],
                rhs=pe_t[:, oc * 1024 + u * 512 : oc * 1024 + (u + 1) * 512],
                start=False,
                stop=True,
            )

    # Tail: query half B accumulates in the S-pool banks (free once the last
    # exp has read them) so it does NOT wait on the half-A flush below.
    ob_tiles = []
    for _oc in range(2):
        O_b = spsum.tile([P, 1024], F32, tag="S")
        ob_tiles.append(O_b)
    prev_mm = None
    for kt in range(KT_TILES):
        for oc in range(2):
            for u in range(2):
                mm = nc.tensor.matmul(
                    ob_tiles[oc][:, u * 512 : (u + 1) * 512],
                    lhsT=v_tiles[kt][:],
                    rhs=e_tiles[kt][:, 2048 + oc * 1024 + u * 512 : 2048 + oc * 1024 + (u + 1) * 512],
                    start=(kt == 0),
                    stop=(kt == KT_TILES - 1),
                )
                # chain ordering so each vsc weight-load run stays a
                # contiguous block of 4, and so the half-A close runs FIRST
                # (otherwise the scheduler defers it to the very end and the
                # half-A flush + DMAs pile up behind the half-B tail)
                add_dep_helper(
                    mm.ins,
                    (prev_mm or last_close).ins,
                    sync=False,
                    reason="contiguous vsc weight runs in tail",
                )
                prev_mm = mm

    # Flushes: fp16 staging (halves the out DMA bytes; the host adds the two
    # per-batch partials in fp32).  Copies split across ACT (idle in the
    # tail) and DVE in 512-col quarters so the last copy->DMA chain is short;
    # each quarter's DMA issues as soon as it is staged.
    def flush(o_pair, lo):
        for oc in range(2):
            for qr in range(2):
                src = o_pair[oc][:, qr * 512 : (qr + 1) * 512]
                dst = oacc[:, lo + oc * 1024 + qr * 512 : lo + oc * 1024 + (qr + 1) * 512]
                if oc == 0:
                    nc.scalar.copy(out=dst, in_=src)
                else:
                    nc.vector.tensor_copy(out=dst, in_=src)
                nc.sync.dma_start(
                    out_d[:, lo + oc * 1024 + qr * 512 : lo + oc * 1024 + (qr + 1) * 512],
                    dst,
                )

    flush(oa_tiles, 0)
    flush(ob_tiles, 2048)


def build_bass(repeat=1, loop=False):
    nc = bass.Bass("TRN2", target_bir_lowering=False, debug=False)
    qt_d = nc.dram_tensor("qt", [P, N], F16, kind="ExternalInput").ap()
    kt_d = nc.dram_tensor("kt", [P, NK], F16, kind="ExternalInput").ap()
    v_d = nc.dram_tensor("v", [NK, D], F16, kind="ExternalInput").ap()
    out_d = nc.dram_tensor("out_t", [P, N], F16, kind="ExternalOutput").ap()

    with tile.TileContext(nc) as tc:
        with (
            tc.tile_pool(name="big", bufs=1) as big,
            tc.tile_pool(name="epool", bufs=1) as epool,
            tc.tile_pool(name="small", bufs=2) as small,
            tc.tile_pool(name="spsum", bufs=2, space="PSUM") as spsum,
            tc.tile_pool(name="opsum", bufs=2, space="PSUM") as opsum,
        ):
            def body():
                emit_body(
                    nc,
                    tc,
                    (big, epool, small, spsum, opsum),
                    (qt_d, kt_d, v_d, out_d),
                )

            if loop and repeat > 1:
                with tc.For_i(
                    0, repeat, 1,
                    hint_engines=(mybir.EngineType.PE, mybir.EngineType.Activation),
                ):
                    body()
            else:
                for _ in range(repeat):
                    body()
    return nc


def legalize_waits(nc, max_waits=1):
    """Hoist excess semaphore waits into standalone EventSemaphore ops.

    The walrus codegen for several engine instruction structs accepts only a
    single sync-wait command; Tile sometimes emits more.  Executing the extra
    waits in a preceding same-engine EventSemaphore is semantically identical
    (the engine runs its stream in order).
    """
    for fn in nc.m.functions:
        for blk in fn.blocks:
            out = []
            for inst in blk.instructions:
                si = inst.sync_info
                if (
                    si is not None
                    and si.on_wait
                    and len(si.on_wait) > max_waits
                    and inst.opcode != "EventSemaphore"
                ):
                    waits = list(si.on_wait)
                    extra, keep = waits[:-max_waits], waits[-max_waits:]
                    for n, w in enumerate(extra):
                        out.append(
                            mybir.InstEventSemaphore(
                                name=f"{inst.name}_prewait{n}",
                                engine=inst.engine,
                                ins=[],
                                outs=[],
                                sync_info=mybir.SyncInfo(on_wait=[w], on_update=[]),
                            )
                        )
                    si.on_wait = keep
                out.append(inst)
            blk.instructions = out
    return nc


_NC_CACHE = {}


def _get_nc(repeat=1, **kw):
    key = ("nc", repeat, tuple(sorted(kw.items())))
    if key not in _NC_CACHE:
        _NC_CACHE[key] = legalize_waits(build_bass(repeat, **kw))
    return _NC_CACHE[key]


def kernel(q, k, v):
    q = np.asarray(q, dtype=np.float32)
    k = np.asarray(k, dtype=np.float32)
    v = np.asarray(v, dtype=np.float32)

    in_maps = []
    for c in range(8):
        b, h = c // 2, c % 2
        in_maps.append(
            {
                "qt": np.ascontiguousarray(q[b].T).astype(np.float16),
                "kt": np.ascontiguousarray(k[b, h * NK : (h + 1) * NK].T).astype(np.float16),
                "v": np.ascontiguousarray(v[b, h * NK : (h + 1) * NK]).astype(np.float16),
            }
        )

    nc = _get_nc()
    res = run_bass_kernel_spmd(nc, in_maps, list(range(8))).results

    out = np.empty((B, N, D), dtype=np.float32)
    for b in range(B):
        out[b] = (
            res[2 * b]["out_t"].astype(np.float32)
            + res[2 * b + 1]["out_t"].astype(np.float32)
        ).T
    return out


# revision 25
# speedup vs baseline: 1.1344x; 1.1142x over previous
"""Trainium2 Bass kernel for column-softmax attention.

reference semantics:
    scores = einsum('bqd,bkd->bqk', q, k) / sqrt(128)   # [B, Nq, Nk]
    attn   = softmax(scores, axis=1)                     # over the QUERY axis
    out    = einsum('bqk,bkd->bqd', attn, v)             # [B, Nq, D]

Because the softmax is over q, each key column k normalizes independently:
    out[q, d] = sum_k E[k, q] * r[k] * v[k, d],  E = exp(scores.T), r = 1/sum_q E[k, q]

Sharding: 8 cores = 4 batches x 2 key-halves.  Each core computes the partial
sum over its 2048 keys; the host adds the two partials per batch.

On-chip layout: the host pre-transposes Q and K to [D, N] (contraction dim on
partitions) and the kernel emits out.T [D, Nq]; the host transposes back.  The
softmax denominator is folded into V row-scaling so the normalize step touches
only 128x128 tiles per key tile.

The ScalarE exp pass (8.4M elements/core) is the roofline: 64 ACTIVATE
instructions of N=1024 from PSUM = ~66us engine-busy, the largest chunk the
8-bank PSUM allows while keeping a double-buffered score tile (4 banks) AND a
query-half output accumulator (4 banks) resident.  Structure:
  - row sums moved OFF ScalarE (accum_out costs ~190-280ns per ACTIVATE):
    per key tile a 3-stage fp16 pairwise-add chain + short reduce on DVE
    (~2.9us, fits under the 4.15us ACT tile period).  Last tile keeps
    accum_out so vsc15 is ready immediately for the tail.
  - startup: first q chunk + first key tile DMA'd first; a dummy exp preloads
    the ACT table and a 5-matmul memset warm-up stream releases the PE HAM
    clock-gate, all under the initial DMA window.
  - phase A (per key tile): scores matmul -> exp -> E resident fp16; previous
    tile's out.T contribution for query half A accumulated in PSUM (hides the
    second GEMM under the ScalarE exp span).
  - tail: query half B accumulated in the freed S-pool PSUM banks (so it does
    not wait on the half-A flush); half-A close ordered before the half-B
    stream; flush copies in 512-col quarters split ACT/DVE, fp16 staging
    halves the out DMA, each quarter DMAs as soon as staged.
PE weight-load runs are kept contiguous per stationary via explicit
ordering-only deps (the scheduler would otherwise split runs).
Measured: TimelineSim 110.8us baseline -> 92.9us.  Axon loop-difference
(back-to-back A/B, R=257 sustained): baseline 110.1us -> 94.6us; quiet-window
R=129 burst readings ~79-81us vs the 95.3us baseline quote (~14-16% either way).
"""

import numpy as np

import concourse.bass as bass
import concourse.mybir as mybir
import concourse.tile as tile
from concourse.bass_utils import run_bass_kernel_spmd
from concourse.tile_rust import add_dep_helper

B, N, D = 4, 4096, 128
P = 128
NK = 2048                 # keys per core (half of 4096)
KT_TILES = NK // P        # 16 key tiles of 128
SCALE = 1.0 / np.sqrt(128.0)

F32 = mybir.dt.float32
F16 = mybir.dt.float16


def emit_body(nc, tc, pools, aps):
    big, epool, small, spsum, opsum = pools
    qt_d, kt_d, v_d, out_d = aps

    qT = big.tile([P, N], F16, tag="qT")            # [d, q]
    kT = big.tile([P, NK], F16, tag="kT")           # [d, k]
    vsb = big.tile([P, KT_TILES, D], F16, tag="v")  # [k_in_tile, k_tile, d]
    oacc = big.tile([P, N], F16, tag="oacc")        # [d, q] fp16 staging

    # DMA order: the first scores matmul needs kT tile 0 + qT[0:512] only.
    nc.sync.dma_start(qT[:, 0:512], qt_d[:, 0:512])
    nc.sync.dma_start(kT[:, 0:P], kt_d[:, 0:P])
    nc.sync.dma_start(qT[:, 512:1024], qt_d[:, 512:1024])
    for _qc in range(1, 4):
        nc.sync.dma_start(
            qT[:, _qc * 1024 : (_qc + 1) * 1024], qt_d[:, _qc * 1024 : (_qc + 1) * 1024]
        )
    # kT tiles 1.. are first needed at tile-1 scores (~7us in), q chunks first.
    nc.sync.dma_start(kT[:, P:NK], kt_d[:, P:NK])
    nc.sync.dma_start(vsb[:], v_d.rearrange("(t p) d -> p t d", p=P))

    # ACT table preload: dummy exp on a memset tile runs under the DMA window,
    # so the ~1.3us exp_and_others table load is off the critical path.
    warm_in = small.tile([P, 1], F32, tag="warm_in")
    warm_out = small.tile([P, 1], F32, tag="warm_out")
    nc.vector.memset(warm_in[:], 0.0)
    nc.scalar.activation(
        out=warm_out[:], in_=warm_in[:], func=mybir.ActivationFunctionType.Exp
    )

    # PE warm-up: a stream of matmuls on memset data runs during the
    # otherwise-idle DMA window, so the HAM clock-gate (cold 1.2GHz ->
    # warm 2.4GHz after ~3.4us of sustained PE activity) is released
    # before the first real scores matmul arrives.
    wsrc = big.tile([P, 512], F16, tag="wsrc")
    nc.vector.memset(wsrc[:], 0.0)
    Swarm = spsum.tile([P, 1024], F32, tag="S")
    for _w in range(5):
        nc.tensor.matmul(
            Swarm[:, 0:512], lhsT=wsrc[:, 0:P], rhs=wsrc[:], start=True, stop=True
        )

    e_tiles = []
    v_tiles = []
    # Output accumulators for query half A (cols 0..2047) are built up during
    # phase A so the second GEMM's first half hides under the exp span.
    oa_tiles = []
    for _oc in range(2):
        O_a = opsum.tile([P, 1024], F32, tag="O")
        oa_tiles.append(O_a)

    pending_g2a = None
    for kt in range(KT_TILES):
        last_tile = kt == KT_TILES - 1
        E = epool.tile([P, N], F16, tag=f"E{kt}")   # [k, q] = exp(scores.T)
        rs = small.tile([P, 4], F32, tag="rs")
        last_g1 = None
        for h in range(4):
            S = spsum.tile([P, 1024], F32, tag="S")
            for u in range(2):
                last_g1 = nc.tensor.matmul(
                    S[:, u * 512 : (u + 1) * 512],
                    lhsT=kT[:, kt * P : (kt + 1) * P],
                    rhs=qT[:, h * 1024 + u * 512 : h * 1024 + u * 512 + 512],
                    start=True,
                    stop=True,
                )
            # For the LAST tile only, fuse the row sums into the activation
            # (accum_out): the DVE reduce chain would otherwise put ~3us of
            # vsc15 latency on the critical path into the tail.  For all
            # other tiles the fused accumulator read costs ~190-280ns of
            # ScalarE per chunk, so the row sums go to DVE instead.
            nc.scalar.activation(
                out=E[:, h * 1024 : (h + 1) * 1024],
                in_=S[:],
                func=mybir.ActivationFunctionType.Exp,
                scale=float(SCALE),
                accum_out=rs[:, h : h + 1] if last_tile else None,
            )
        rsum = small.tile([P, 1], F32, tag="rsum")
        recip = small.tile([P, 1], F32, tag="recip")
        vsc = small.tile([P, D], F16, tag=f"vsc{kt}")  # [k, d] * r[k]
        if last_tile:
            nc.vector.reduce_sum(out=rsum[:], in_=rs[:], axis=mybir.AxisListType.X)
        else:
            # Row sums on DVE, off the ScalarE path.  A straight TensorReduce
            # over [128, 4096] runs at 1 elem/cycle = ~4.3us/tile and would
            # make DVE the per-tile bottleneck; fp16 pairwise adds get the
            # 2x DVE rate, so fold 4096 -> 512 in three adds + short reduce
            # (~2.9us).  (tensor_tensor_reduce would be one instruction but
            # does not compile on this toolchain: "ISA wrong length".)
            t1 = small.tile([P, 2048], F16, tag="rt1")
            t2 = small.tile([P, 1024], F16, tag="rt2")
            t3 = small.tile([P, 512], F16, tag="rt3")
            nc.vector.tensor_add(t1[:], E[:, 0:2048], E[:, 2048:4096])
            nc.vector.tensor_add(t2[:], t1[:, 0:1024], t1[:, 1024:2048])
            nc.vector.tensor_add(t3[:], t2[:, 0:512], t2[:, 512:1024])
            nc.vector.reduce_sum(out=rsum[:], in_=t3[:], axis=mybir.AxisListType.X)
        nc.vector.reciprocal(recip[:], rsum[:])
        nc.vector.tensor_scalar_mul(vsc[:], vsb[:, kt, :], recip[:])
        e_tiles.append(E)
        v_tiles.append(vsc)

        # Emit the PREVIOUS key tile's half-A output matmuls here, ordered
        # after this tile's scores matmuls (ordering-only deps).  This keeps
        # each kT weight-load run contiguous.
        if kt > 0:
            pv, pe_t, pkt = pending_g2a
            for oc in range(2):
                for u in range(2):
                    mm = nc.tensor.matmul(
                        oa_tiles[oc][:, u * 512 : (u + 1) * 512],
                        lhsT=pv[:],
                        rhs=pe_t[:, oc * 1024 + u * 512 : oc * 1024 + (u + 1) * 512],
                        start=(pkt == 0),
                        stop=False,
                    )
                    if last_g1 is not None:
                        add_dep_helper(
                            mm.ins,
                            last_g1.ins,
                            sync=False,
                            reason="keep kT weight-load run contiguous",
                        )
        pending_g2a = (vsc, E, kt)

    # Close the half-A accumulation with the last key tile's contribution.
    pv, pe_t, pkt = pending_g2a
    last_close = None
    for oc in range(2):
        for u in range(2):
            last_close = nc.tensor.matmul(
                oa_tiles[oc][:, u * 512 : (u + 1) * 512],
                lhsT=pv[:],
                rhs=pe_t[:, oc * 1024 + u * 512 : oc * 1024 + (u + 1) * 512],
                start=False,
                stop=True,
            )

    # Flush one [128, 1024] accumulator: two 512-col copies split across ACT
    # (idle in the tail) and DVE, each quarter DMA'd as soon as it is staged.
    # fp16 staging halves the out DMA bytes (the host adds the two per-batch
    # partials in fp32).
    def flush_oc(o_tile, lo):
        for qr in range(2):
            src = o_tile[:, qr * 512 : (qr + 1) * 512]
            dst = oacc[:, lo + qr * 512 : lo + (qr + 1) * 512]
            if qr == 0:
                nc.scalar.copy(out=dst, in_=src)
            else:
                nc.vector.tensor_copy(out=dst, in_=src)
            nc.sync.dma_start(out_d[:, lo + qr * 512 : lo + (qr + 1) * 512], dst)

    # Tail: query half B accumulates in the S-pool banks (free once the last
    # exp has read them) so it does NOT wait on the half-A flush.  The two
    # 1024-col output groups run SEQUENTIALLY (oc0's 32 matmuls over all 16
    # key tiles, then oc1's) so oc0's flush copies and DMAs hide completely
    # under oc1's matmul stream and only oc1's short flush chain remains at
    # the very end.  Costs one extra vsc weight-load run per tile (hidden
    # under the matmul stream).  The half-A close is chained FIRST so its
    # flush + DMAs issue early instead of piling up behind the tail.
    ob_tiles = []
    for _oc in range(2):
        O_b = spsum.tile([P, 1024], F32, tag="S")
        ob_tiles.append(O_b)
    prev_mm = last_close
    # Group order alternates the two PSUM tiles so each quarter's flush copy
    # (a read of the tile) finishes under a DIFFERENT tile's matmul stream --
    # same-tile write-after-read would stall the next group ~0.9us.
    for qt4, (oc, u) in enumerate([(0, 0), (1, 0), (0, 1), (1, 1)]):
        for kt in range(KT_TILES):
            mm = nc.tensor.matmul(
                ob_tiles[oc][:, u * 512 : (u + 1) * 512],
                lhsT=v_tiles[kt][:],
                rhs=e_tiles[kt][:, 2048 + oc * 1024 + u * 512 : 2048 + oc * 1024 + (u + 1) * 512],
                start=(kt == 0),
                stop=(kt == KT_TILES - 1),
            )
            add_dep_helper(
                mm.ins, prev_mm.ins, sync=False,
                reason="sequential bank groups in tail",
            )
            prev_mm = mm
        # This quarter's flush + DMA hide under the next quarter's matmuls.
        src = ob_tiles[oc][:, u * 512 : (u + 1) * 512]
        lo = 2048 + oc * 1024 + u * 512
        dst = oacc[:, lo : lo + 512]
        if qt4 % 2 == 0:
            nc.scalar.copy(out=dst, in_=src)
        else:
            nc.vector.tensor_copy(out=dst, in_=src)
        nc.sync.dma_start(out_d[:, lo : lo + 512], dst)
        if qt4 == 0:
            flush_oc(oa_tiles[0], 0)
            flush_oc(oa_tiles[1], 1024)


def build_bass(repeat=1, loop=False):
    nc = bass.Bass("TRN2", target_bir_lowering=False, debug=False)
    qt_d = nc.dram_tensor("qt", [P, N], F16, kind="ExternalInput").ap()
    kt_d = nc.dram_tensor("kt", [P, NK], F16, kind="ExternalInput").ap()
    v_d = nc.dram_tensor("v", [NK, D], F16, kind="ExternalInput").ap()
    out_d = nc.dram_tensor("out_t", [P, N], F16, kind="ExternalOutput").ap()

    with tile.TileContext(nc) as tc:
        with (
            tc.tile_pool(name="big", bufs=1) as big,
            tc.tile_pool(name="epool", bufs=1) as epool,
            tc.tile_pool(name="small", bufs=2) as small,
            tc.tile_pool(name="spsum", bufs=2, space="PSUM") as spsum,
            tc.tile_pool(name="opsum", bufs=2, space="PSUM") as opsum,
        ):
            def body():
                emit_body(
                    nc,
                    tc,
                    (big, epool, small, spsum, opsum),
                    (qt_d, kt_d, v_d, out_d),
                )

            if loop and repeat > 1:
                with tc.For_i(
                    0, repeat, 1,
                    hint_engines=(mybir.EngineType.PE, mybir.EngineType.Activation),
                ):
                    body()
            else:
                for _ in range(repeat):
                    body()
    return nc


def legalize_waits(nc, max_waits=1):
    """Hoist excess semaphore waits into standalone EventSemaphore ops.

    The walrus codegen for several engine instruction structs accepts only a
    single sync-wait command; Tile sometimes emits more.  Executing the extra
    waits in a preceding same-engine EventSemaphore is semantically identical
    (the engine runs its stream in order).
    """
    for fn in nc.m.functions:
        for blk in fn.blocks:
            out = []
            for inst in blk.instructions:
                si = inst.sync_info
                if (
                    si is not None
                    and si.on_wait
                    and len(si.on_wait) > max_waits
                    and inst.opcode != "EventSemaphore"
                ):
                    waits = list(si.on_wait)
                    extra, keep = waits[:-max_waits], waits[-max_waits:]
                    for n, w in enumerate(extra):
                        out.append(
                            mybir.InstEventSemaphore(
                                name=f"{inst.name}_prewait{n}",
                                engine=inst.engine,
                                ins=[],
                                outs=[],
                                sync_info=mybir.SyncInfo(on_wait=[w], on_update=[]),
                            )
                        )
                    si.on_wait = keep
                out.append(inst)
            blk.instructions = out
    return nc


_NC_CACHE = {}


def _get_nc(repeat=1, **kw):
    key = ("nc", repeat, tuple(sorted(kw.items())))
    if key not in _NC_CACHE:
        _NC_CACHE[key] = legalize_waits(build_bass(repeat, **kw))
    return _NC_CACHE[key]


def kernel(q, k, v):
    q = np.asarray(q, dtype=np.float32)
    k = np.asarray(k, dtype=np.float32)
    v = np.asarray(v, dtype=np.float32)

    in_maps = []
    for c in range(8):
        b, h = c // 2, c % 2
        in_maps.append(
            {
                "qt": np.ascontiguousarray(q[b].T).astype(np.float16),
                "kt": np.ascontiguousarray(k[b, h * NK : (h + 1) * NK].T).astype(np.float16),
                "v": np.ascontiguousarray(v[b, h * NK : (h + 1) * NK]).astype(np.float16),
            }
        )

    nc = _get_nc()
    res = run_bass_kernel_spmd(nc, in_maps, list(range(8))).results

    out = np.empty((B, N, D), dtype=np.float32)
    for b in range(B):
        out[b] = (
            res[2 * b]["out_t"].astype(np.float32)
            + res[2 * b + 1]["out_t"].astype(np.float32)
        ).T
    return out
